# revision 14
# baseline (speedup 1.0000x reference)
"""Per-sample batched matmul: out[b,o,f] = sum_i weights[b,o,i] * x[b,i,f].

Sharding: batch (bs=32) split across 8 NeuronCores, 4 samples each, zero
communication. Per sample the kernel PE-transposes W_b (fp32, exact) into
[I, O] layout, then runs accumulating float32r matmuls with x_b as the
moving operand (f32r = fp32 inputs truncated in the PE; accumulation
stays fp32 in PSUM). bf16 weights were tried and are SLOWER: the fast-
weight-load path they enable steals XBUS rails from the moving stream
(measured 259ns/matmul vs f32r's 227ns), while f32r's slower LDWEIGHTS
overlaps almost entirely.

v5 structure:
- software-pipelined emission: sample b+1's W pipeline (DMA on the sync
  ring -> PE transposes -> DVE/ACT eviction into the [I,O] stationary
  layout) is emitted between sample b's matmul groups, so per-engine
  FIFO order never stalls the PE at a sample boundary.
- x chunks: one dma_start per 1MB chunk on the scalar ring (each issue
  costs ~0.7us of engine time; splits land on the same ring anyway),
  issued 2 chunks ahead.
- device output in bf16 (host upcasts): halves output HBM traffic so
  input streams never contend with the ~358 GB/s per-core HBM budget,
  and shortens the drain tail; outputs ride GpSimd/SWDGE except the
  last groups which use the by-then-idle sync ring.
"""

import sys

try:  # concourse (Bass/Tile) ships in the container, not on default sys.path
    import concourse  # noqa: F401
except ImportError:
    sys.path.insert(0, "/opt/trn_rl_repo")

import numpy as np

BS, IN_SIZE, OUT_SIZE, FEATS = 32, 1024, 1024, 2048
N_CORES = 8
BPC = BS // N_CORES  # samples per core

P = 128
N_FREE = 512  # moving-operand free dim per matmul (1 PSUM bank of fp32)
KO = IN_SIZE // P  # 8 contraction tiles
MO = OUT_SIZE // P  # 8 output-row tiles
NF = FEATS // N_FREE  # 4 output-col chunks
NCHUNK = BPC * NF  # 16 x-chunks, processed in order

_NC_CACHE = {}


def _build_nc():
    import concourse.mybir as mybir
    import concourse.tile as tile
    from concourse import bacc

    f32 = mybir.dt.float32
    f32r = mybir.dt.float32r
    bf16 = mybir.dt.bfloat16

    nc = bacc.Bacc("TRN2", target_bir_lowering=False, debug=False)
    x_d = nc.dram_tensor(
        "x", [BPC, IN_SIZE, FEATS], f32, kind="ExternalInput"
    ).ap()
    w_d = nc.dram_tensor(
        "w", [BPC, OUT_SIZE, IN_SIZE], f32, kind="ExternalInput"
    ).ap()
    o_d = nc.dram_tensor(
        "out", [BPC, OUT_SIZE, FEATS], bf16, kind="ExternalOutput"
    ).ap()

    with tile.TileContext(nc) as tc:
        with (
            tc.tile_pool(name="const", bufs=1) as const,
            tc.tile_pool(name="wn_pool", bufs=12) as wn_pool,
            tc.tile_pool(name="wt_pool", bufs=2) as wt_pool,
            tc.tile_pool(name="xn_pool", bufs=5) as xn_pool,
            tc.tile_pool(name="ot_pool", bufs=6) as ot_pool,
            tc.tile_pool(name="psmm", bufs=5, space="PSUM") as psmm_pool,
            tc.tile_pool(name="pstr", bufs=3, space="PSUM") as pstr_pool,
        ):
            eye_d = nc.inline_tensor(np.eye(P, dtype=np.float32), name="eye")
            ident = const.tile([P, P], f32r, name="identr")
            nc.sync.dma_start(ident[:], eye_d.ap().bitcast(f32r))

            TG = 4  # transposes packed per PSUM bank

            # alternate DVE/ACT for every eviction so neither engine's
            # FIFO becomes the critical path
            par = {"i": 0}

            def alt_copy(dst, src):
                par["i"] += 1
                if par["i"] % 2 == 0:
                    nc.vector.tensor_copy(out=dst, in_=src)
                else:
                    nc.scalar.copy(dst, src)

            xr = [x_d[b].rearrange("(ko p) f -> p ko f", p=P) for b in range(BPC)]
            xn = {}  # chunk -> f32r x tile
            wn = {}  # (b, mo) -> f32r W row-block
            wt = {}  # b -> [P, KO, MO, P] f32r stationary layout

            def issue_xdma(k):
                b, n = divmod(k, NF)
                t = xn_pool.tile([P, KO, N_FREE], f32r, tag="xn", name=f"xn_{k}")
                nc.scalar.dma_start(
                    t[:], xr[b][:, :, n * N_FREE : (n + 1) * N_FREE].bitcast(f32r)
                )
                xn[k] = t

            def issue_wdma(b, mo, ways=1):
                t = wn_pool.tile([P, IN_SIZE], f32r, tag="wn", name=f"wn_{b}_{mo}")
                src = w_d[b, mo * P : (mo + 1) * P, :].bitcast(f32r)
                w = IN_SIZE // ways
                for q in range(ways):
                    nc.sync.dma_start(
                        t[:, q * w : (q + 1) * w], src[:, q * w : (q + 1) * w]
                    )
                wn[(b, mo)] = t

            def emit_w_station(b, mo):
                """8 f32r transposes of one W row-block; 4 share a PSUM bank,
                each bank leaves via one wide copy, alternating DVE/ACT."""
                src = wn.pop((b, mo))
                for g in range(KO // TG):
                    pt = pstr_pool.tile(
                        [P, TG * P], f32r, tag="pt", name=f"pt_{b}_{mo}_{g}"
                    )
                    for c in range(TG):
                        ko = g * TG + c
                        nc.tensor.transpose(
                            pt[:, c * P : (c + 1) * P],
                            src[:, ko * P : (ko + 1) * P],
                            ident[:],
                        )
                    alt_copy(
                        wt[b][:, g * TG : (g + 1) * TG, mo, :],
                        pt[:].rearrange("p (c q) -> p c q", c=TG),
                    )

            def mm_group(k, mo):
                """One [128, 512] output tile: 8 accumulating matmuls, a
                cast-evict to bf16, and an output DMA on GpSimd (SWDGE) so
                compute-lagged output waits never block input prefetch. The
                final groups instead split the evict across DVE+ACT and
                drain on the (idle by then) sync ring for a shorter tail."""
                b, n = divmod(k, NF)
                xt = xn[k]
                ps = psmm_pool.tile([P, N_FREE], f32, tag="ps", name=f"ps_{k}_{mo}")
                for ko in range(KO):
                    nc.tensor.matmul(
                        ps[:],
                        wt[b][:, ko, mo, :],
                        xt[:, ko, :],
                        start=(ko == 0),
                        stop=(ko == KO - 1),
                    )
                ot = ot_pool.tile([P, N_FREE], bf16, tag="ot", name=f"ot_{k}_{mo}")
                dst = o_d[b, mo * P : (mo + 1) * P, n * N_FREE : (n + 1) * N_FREE]
                if k == NCHUNK - 1 and mo == MO - 1:
                    h = N_FREE // 2
                    nc.vector.tensor_copy(out=ot[:, :h], in_=ps[:, :h])
                    nc.scalar.copy(ot[:, h:], ps[:, h:])
                    nc.sync.dma_start(dst[:, :h], ot[:, :h])
                    nc.sync.dma_start(dst[:, h:], ot[:, h:])
                    return
                alt_copy(ot[:], ps[:])
                if k == NCHUNK - 1 and mo == MO - 2:
                    nc.sync.dma_start(dst, ot[:])
                else:
                    nc.gpsimd.dma_start(dst, ot[:])

            # ---- HAM warmup: ~3.4us of identity transposes while the first
            # DMAs are in flight, so the real work starts on a warm PE.
            warm_sink = const.tile([P, 16], f32r, name="warm_sink")
            for wg in range(8):
                ptw = pstr_pool.tile([P, TG * P], f32r, tag="pt", name=f"ptw_{wg}")
                for c in range(TG):
                    nc.tensor.transpose(
                        ptw[:, c * P : (c + 1) * P], ident[:], ident[:]
                    )
                nc.vector.tensor_copy(out=warm_sink[:], in_=ptw[:, :16])

            for b in range(BPC):
                wt[b] = wt_pool.tile(
                    [P, KO, MO, P], f32r, tag="wt", name=f"wt_{b}"
                )

            # ---- startup: sample 0's W pipeline interleaves with its first
            # chunk's matmul groups, paced by the arriving DMAs.
            issue_xdma(0)
            issue_xdma(1)
            issue_wdma(0, 0, ways=2)
            issue_wdma(0, 1, ways=2)
            for mo in range(2, MO):
                issue_wdma(0, mo)
            emit_w_station(0, 0)
            for mo in range(MO):
                mm_group(0, mo)
                if mo + 1 < MO:
                    emit_w_station(0, mo + 1)
                if mo == 6:
                    issue_xdma(2)

            # ---- steady state: chunk k runs its 8 groups; meanwhile chunk
            # k+2's DMA is issued and (during local chunks n=1,2) sample
            # b+1's W stations are emitted between groups.
            for k in range(1, NCHUNK):
                b, n = divmod(k, NF)
                if k + 2 < NCHUNK:
                    issue_xdma(k + 2)
                for mo in range(MO):
                    mm_group(k, mo)
                    if n in (1, 2) and b + 1 < BPC:
                        j = (n - 1) * (MO // 2) + mo // 2
                        if mo % 2 == 0:
                            issue_wdma(b + 1, j)
                        else:
                            emit_w_station(b + 1, j)

    nc.compile()
    return nc


def run(x, weights, trace=False):
    """Shard on batch, run SPMD on 8 cores, gather. Returns (out, results)."""
    from concourse.bass_utils import run_bass_kernel_spmd

    key = "nc"
    if key not in _NC_CACHE:
        _NC_CACHE[key] = _build_nc()
    nc = _NC_CACHE[key]

    x = np.ascontiguousarray(np.asarray(x, dtype=np.float32))
    weights = np.ascontiguousarray(np.asarray(weights, dtype=np.float32))
    in_maps = [
        {
            "x": x[c * BPC : (c + 1) * BPC],
            "w": weights[c * BPC : (c + 1) * BPC],
        }
        for c in range(N_CORES)
    ]
    last_err = None
    for attempt in range(3):
        try:
            res = run_bass_kernel_spmd(
                nc, in_maps, core_ids=list(range(N_CORES)), trace=trace
            )
            break
        except Exception as e:  # transient NRT device faults: back off, retry
            last_err = e
            import time as _time

            _time.sleep(5 * (attempt + 1))
    else:
        raise last_err
    out = np.concatenate(
        [
            np.asarray(res.results[c]["out"]).astype(np.float32)
            for c in range(N_CORES)
        ],
        axis=0,
    )
    return out, res


def kernel(x, weights):
    out, _ = run(x, weights, trace=False)
    return out


# revision 15
# speedup vs baseline: 1.0254x; 1.0254x over previous
"""Per-sample batched matmul: out[b,o,f] = sum_i weights[b,o,i] * x[b,i,f].

Sharding: batch (bs=32) split across 8 NeuronCores, 4 samples each, zero
communication. Per sample the kernel PE-transposes W_b (fp32, exact) into
[I, O] layout, then runs accumulating float32r matmuls with x_b as the
moving operand (f32r = fp32 inputs truncated in the PE; accumulation
stays fp32 in PSUM). bf16 weights were tried and are SLOWER: the fast-
weight-load path they enable steals XBUS rails from the moving stream
(measured 259ns/matmul vs f32r's 227ns), while f32r's slower LDWEIGHTS
overlaps almost entirely.

v5 structure:
- software-pipelined emission: sample b+1's W pipeline (DMA on the sync
  ring -> PE transposes -> DVE/ACT eviction into the [I,O] stationary
  layout) is emitted between sample b's matmul groups, so per-engine
  FIFO order never stalls the PE at a sample boundary.
- x chunks: one dma_start per 1MB chunk on the scalar ring (each issue
  costs ~0.7us of engine time; splits land on the same ring anyway),
  issued 2 chunks ahead.
- device output in bf16 (host upcasts): halves output HBM traffic so
  input streams never contend with the ~358 GB/s per-core HBM budget,
  and shortens the drain tail; outputs ride GpSimd/SWDGE except the
  last groups which use the by-then-idle sync ring.
"""

import sys

try:  # concourse (Bass/Tile) ships in the container, not on default sys.path
    import concourse  # noqa: F401
except ImportError:
    sys.path.insert(0, "/opt/trn_rl_repo")

import numpy as np

BS, IN_SIZE, OUT_SIZE, FEATS = 32, 1024, 1024, 2048
N_CORES = 8
BPC = BS // N_CORES  # samples per core

P = 128
N_FREE = 512  # moving-operand free dim per matmul (1 PSUM bank of fp32)
KO = IN_SIZE // P  # 8 contraction tiles
MO = OUT_SIZE // P  # 8 output-row tiles
NF = FEATS // N_FREE  # 4 output-col chunks
NCHUNK = BPC * NF  # 16 x-chunks, processed in order

_NC_CACHE = {}


def _build_nc():
    import concourse.mybir as mybir
    import concourse.tile as tile
    from concourse import bacc

    f32 = mybir.dt.float32
    f32r = mybir.dt.float32r
    bf16 = mybir.dt.bfloat16

    nc = bacc.Bacc("TRN2", target_bir_lowering=False, debug=False)
    x_d = nc.dram_tensor(
        "x", [BPC, IN_SIZE, FEATS], f32, kind="ExternalInput"
    ).ap()
    w_d = nc.dram_tensor(
        "w", [BPC, OUT_SIZE, IN_SIZE], f32, kind="ExternalInput"
    ).ap()
    o_d = nc.dram_tensor(
        "out", [BPC, OUT_SIZE, FEATS], bf16, kind="ExternalOutput"
    ).ap()

    with tile.TileContext(nc) as tc:
        with (
            tc.tile_pool(name="const", bufs=1) as const,
            tc.tile_pool(name="wn_pool", bufs=12) as wn_pool,
            tc.tile_pool(name="wt_pool", bufs=2) as wt_pool,
            tc.tile_pool(name="xn_pool", bufs=5) as xn_pool,
            tc.tile_pool(name="ot_pool", bufs=12) as ot_pool,
            tc.tile_pool(name="psmm", bufs=5, space="PSUM") as psmm_pool,
            tc.tile_pool(name="pstr", bufs=3, space="PSUM") as pstr_pool,
        ):
            eye_d = nc.inline_tensor(np.eye(P, dtype=np.float32), name="eye")
            ident = const.tile([P, P], f32r, name="identr")
            nc.sync.dma_start(ident[:], eye_d.ap().bitcast(f32r))

            TG = 4  # transposes packed per PSUM bank

            # alternate DVE/ACT for every eviction so neither engine's
            # FIFO becomes the critical path
            par = {"i": 0}

            def alt_copy(dst, src):
                par["i"] += 1
                if par["i"] % 2 == 0:
                    nc.vector.tensor_copy(out=dst, in_=src)
                else:
                    nc.scalar.copy(dst, src)

            xr = [x_d[b].rearrange("(ko p) f -> p ko f", p=P) for b in range(BPC)]
            xn = {}  # chunk -> f32r x tile
            wn = {}  # (b, mo) -> f32r W row-block
            wt = {}  # b -> [P, KO, MO, P] f32r stationary layout

            def issue_xdma(k):
                b, n = divmod(k, NF)
                t = xn_pool.tile([P, KO, N_FREE], f32r, tag="xn", name=f"xn_{k}")
                nc.scalar.dma_start(
                    t[:], xr[b][:, :, n * N_FREE : (n + 1) * N_FREE].bitcast(f32r)
                )
                xn[k] = t

            def issue_wdma(b, mo, ways=1):
                t = wn_pool.tile([P, IN_SIZE], f32r, tag="wn", name=f"wn_{b}_{mo}")
                src = w_d[b, mo * P : (mo + 1) * P, :].bitcast(f32r)
                w = IN_SIZE // ways
                for q in range(ways):
                    nc.sync.dma_start(
                        t[:, q * w : (q + 1) * w], src[:, q * w : (q + 1) * w]
                    )
                wn[(b, mo)] = t

            def emit_w_station(b, mo):
                """8 f32r transposes of one W row-block; 4 share a PSUM bank,
                each bank leaves via one wide copy, alternating DVE/ACT."""
                src = wn.pop((b, mo))
                for g in range(KO // TG):
                    pt = pstr_pool.tile(
                        [P, TG * P], f32r, tag="pt", name=f"pt_{b}_{mo}_{g}"
                    )
                    for c in range(TG):
                        ko = g * TG + c
                        nc.tensor.transpose(
                            pt[:, c * P : (c + 1) * P],
                            src[:, ko * P : (ko + 1) * P],
                            ident[:],
                        )
                    alt_copy(
                        wt[b][:, g * TG : (g + 1) * TG, mo, :],
                        pt[:].rearrange("p (c q) -> p c q", c=TG),
                    )

            def mm_group(k, mo):
                """One [128, 512] output tile: 8 accumulating matmuls, a
                cast-evict to bf16, and an output DMA on GpSimd (SWDGE) so
                compute-lagged output waits never block input prefetch. The
                final groups instead split the evict across DVE+ACT and
                drain on the (idle by then) sync ring for a shorter tail."""
                b, n = divmod(k, NF)
                xt = xn[k]
                ps = psmm_pool.tile([P, N_FREE], f32, tag="ps", name=f"ps_{k}_{mo}")
                for ko in range(KO):
                    nc.tensor.matmul(
                        ps[:],
                        wt[b][:, ko, mo, :],
                        xt[:, ko, :],
                        start=(ko == 0),
                        stop=(ko == KO - 1),
                    )
                ot = ot_pool.tile([P, N_FREE], bf16, tag="ot", name=f"ot_{k}_{mo}")
                dst = o_d[b, mo * P : (mo + 1) * P, n * N_FREE : (n + 1) * N_FREE]
                if k == NCHUNK - 1 and mo == MO - 1:
                    h = N_FREE // 2
                    nc.vector.tensor_copy(out=ot[:, :h], in_=ps[:, :h])
                    nc.scalar.copy(ot[:, h:], ps[:, h:])
                    nc.sync.dma_start(dst[:, :h], ot[:, :h])
                    nc.sync.dma_start(dst[:, h:], ot[:, h:])
                    return
                alt_copy(ot[:], ps[:])
                if k == NCHUNK - 1 and mo == MO - 2:
                    nc.sync.dma_start(dst, ot[:])
                else:
                    nc.gpsimd.dma_start(dst, ot[:])

            # ---- HAM warmup: ~3.4us of identity transposes while the first
            # DMAs are in flight, so the real work starts on a warm PE.
            warm_sink = const.tile([P, 16], f32r, name="warm_sink")
            for wg in range(8):
                ptw = pstr_pool.tile([P, TG * P], f32r, tag="pt", name=f"ptw_{wg}")
                for c in range(TG):
                    nc.tensor.transpose(
                        ptw[:, c * P : (c + 1) * P], ident[:], ident[:]
                    )
                nc.vector.tensor_copy(out=warm_sink[:], in_=ptw[:, :16])

            for b in range(BPC):
                wt[b] = wt_pool.tile(
                    [P, KO, MO, P], f32r, tag="wt", name=f"wt_{b}"
                )

            # ---- startup: sample 0's W pipeline interleaves with its first
            # chunk's matmul groups, paced by the arriving DMAs.
            issue_xdma(0)
            issue_xdma(1)
            issue_wdma(0, 0, ways=2)
            issue_wdma(0, 1, ways=2)
            for mo in range(2, MO):
                issue_wdma(0, mo)
            emit_w_station(0, 0)
            for mo in range(MO):
                mm_group(0, mo)
                if mo + 1 < MO:
                    emit_w_station(0, mo + 1)
                if mo == 6:
                    issue_xdma(2)

            # ---- steady state: chunk k runs its 8 groups; meanwhile chunk
            # k+2's DMA is issued and (during local chunks n=1,2) sample
            # b+1's W stations are emitted between groups.
            for k in range(1, NCHUNK):
                b, n = divmod(k, NF)
                if k + 2 < NCHUNK:
                    issue_xdma(k + 2)
                for mo in range(MO):
                    mm_group(k, mo)
                    if n in (1, 2) and b + 1 < BPC:
                        j = (n - 1) * (MO // 2) + mo // 2
                        if mo % 2 == 0:
                            issue_wdma(b + 1, j)
                        else:
                            emit_w_station(b + 1, j)

    nc.compile()
    return nc


def run(x, weights, trace=False):
    """Shard on batch, run SPMD on 8 cores, gather. Returns (out, results)."""
    from concourse.bass_utils import run_bass_kernel_spmd

    key = "nc"
    if key not in _NC_CACHE:
        _NC_CACHE[key] = _build_nc()
    nc = _NC_CACHE[key]

    x = np.ascontiguousarray(np.asarray(x, dtype=np.float32))
    weights = np.ascontiguousarray(np.asarray(weights, dtype=np.float32))
    in_maps = [
        {
            "x": x[c * BPC : (c + 1) * BPC],
            "w": weights[c * BPC : (c + 1) * BPC],
        }
        for c in range(N_CORES)
    ]
    last_err = None
    for attempt in range(3):
        try:
            res = run_bass_kernel_spmd(
                nc, in_maps, core_ids=list(range(N_CORES)), trace=trace
            )
            break
        except Exception as e:  # transient NRT device faults: back off, retry
            last_err = e
            import time as _time

            _time.sleep(5 * (attempt + 1))
    else:
        raise last_err
    out = np.concatenate(
        [
            np.asarray(res.results[c]["out"]).astype(np.float32)
            for c in range(N_CORES)
        ],
        axis=0,
    )
    return out, res


def kernel(x, weights):
    out, _ = run(x, weights, trace=False)
    return out
